# revision 1
# baseline (speedup 1.0000x reference)
"""Multi-head attention (B=8, N=1024, C=768, 12 heads) on 8 TRN2 NeuronCores.

Sharding: data-parallel over batch — batch element b runs on core b, weights
replicated, zero collectives.

Per-core kernel (all matmuls bf16 on the TensorEngine):
  - Host pre-transposes x, W_qkv, W_proj so every contraction has its
    reduction axis on SBUF partitions; no on-device transposes needed.
  - qkv: q^T,k^T [768,1024] and v [1024,768] via 6-chunk K=768 matmuls.
  - scores are computed TRANSPOSED per head: S^T[k,q] with lhsT=k^T-block,
    rhs=q^T-block, so the exp output P^T feeds the P@V matmul directly as
    the moving operand (no transpose of the attention matrix). The
    1/sqrt(d) scale rides for free on the exp's affine pre-scale.
  - softmax denominators come free: v is stored with a ones-column
    appended per head (lhsT [128,65]); row 64 of the P@V accumulator is
    sum_k exp(S), i.e. the denominator.
  - normalization runs entirely off the TensorEngine's critical path:
    copy the accumulator to SBUF (releasing its PSUM slot), fast
    approximate reciprocal on VectorE, broadcast across partitions on the
    (otherwise idle) GpSimd engine, one elementwise multiply per head.
  - proj: y = attn @ W_proj^T + b_proj, bias materialized once via
    partition_broadcast and added during the PSUM->SBUF staging.
  - qkv chunk emission is interleaved into the attention stream as
    filler so the TensorEngine stays dense while ScalarE works through
    the exps.
"""

from contextlib import ExitStack

import numpy as np

import concourse.mybir as mybir
import concourse.tile as tile
from concourse import bacc
from concourse.bass_utils import run_bass_kernel_spmd

B, N, C = 8, 1024, 768
NH, D = 12, 64
CK = C // 128  # 6 contraction chunks of 128
NQ = N // 128  # 8 position chunks of 128
SCALE = D ** -0.5
F32 = mybir.dt.float32
BF16 = mybir.dt.bfloat16
Copy = mybir.ActivationFunctionType.Copy
Exp = mybir.ActivationFunctionType.Exp


def _emit(tc, xT, wqkvT, wprojT, bproj, out):
    nc = tc.nc
    with ExitStack() as ctx:
        sb = ctx.enter_context(tc.tile_pool(name="sb", bufs=1))
        stage = ctx.enter_context(tc.tile_pool(name="stage", bufs=12))
        pp = ctx.enter_context(tc.tile_pool(name="pp", bufs=8))
        small = ctx.enter_context(tc.tile_pool(name="small", bufs=2))
        # PSUM pools are released by hand: qkv+attention use ps/acc, the
        # projection reuses the freed banks for a deeper y pipeline.
        ps = tc.alloc_tile_pool(name="ps", bufs=3, space="PSUM")
        acc = tc.alloc_tile_pool(name="acc", bufs=1, space="PSUM")

        # ---- PE warm-up ----------------------------------------------
        # The TensorEngine is idle through the initial DMA lead-in; HAM
        # then starts the first real matmuls at half clock. Keep the PE
        # busy on scratch work so it enters the qkv phase warm.
        warm_in = sb.tile([128, 512], BF16, name="warm_in", tag="warm_in")
        nc.gpsimd.memset(warm_in[:], 1.0)
        warm_ps = ps.tile([128, 512], F32, name="warm_ps", tag="s")
        for i in range(24):
            nc.tensor.matmul(
                warm_ps[:],
                lhsT=warm_in[:, 0:128],
                rhs=warm_in[:],
                start=(i == 0),
                stop=(i == 23),
            )

        # ---- load + bf16-convert x^T and W_qkv^T ---------------------
        # The first scores matmul needs all of x^T plus the q-left and
        # k-left weight columns, so those loads are interleaved per
        # c-chunk; v and the right halves follow.
        xT_bf = [
            sb.tile([128, N], BF16, name=f"xT_bf{c}", tag=f"xT_bf{c}")
            for c in range(CK)
        ]
        wq_bf = [
            sb.tile([128, 3 * C], BF16, name=f"wq_bf{c}", tag=f"wq_bf{c}")
            for c in range(CK)
        ]

        def load_w(g, c, on_act=False):
            w_st = stage.tile([128, 384], F32, name=f"w_st{g}_{c}", tag="stage")
            nc.sync.dma_start(
                out=w_st[:],
                in_=wqkvT[c * 128:(c + 1) * 128, g * 384:(g + 1) * 384],
            )
            # Only the first-needed chunks convert on ScalarE: anything
            # else queued there would sit ahead of the exps in ScalarE's
            # program order and stall the whole attention pipeline.
            if on_act:
                nc.scalar.activation(wq_bf[c][:, g * 384:(g + 1) * 384], w_st[:], Copy)
            else:
                nc.vector.tensor_copy(wq_bf[c][:, g * 384:(g + 1) * 384], w_st[:])

        def load_x(c, qh):
            x_st = stage.tile([128, 512], F32, name=f"x_st{c}_{qh}", tag="stage")
            nc.sync.dma_start(
                out=x_st[:],
                in_=xT[c * 128:(c + 1) * 128, qh * 512:(qh + 1) * 512],
            )
            nc.vector.tensor_copy(xT_bf[c][:, qh * 512:(qh + 1) * 512], x_st[:])

        for c in range(CK):
            load_x(c, 0)
            load_w(0, c, on_act=True)  # q-left
            load_w(2, c, on_act=True)  # k-left
        for c in range(CK):
            load_x(c, 1)

        # ---- qkv projections -----------------------------------------
        # q^T,k^T: chunk m covers rows [m*128,(m+1)*128) of qkv^T;
        # m in 0..5 -> q, m in 6..11 -> k.
        qkT = [
            sb.tile([128, N], BF16, name=f"qkT{m}", tag=f"qkT{m}")
            for m in range(12)
        ]

        def emit_qk(m):
            for qh in range(2):
                qk_ps = ps.tile([128, 512], F32, name=f"qk_ps{m}_{qh}", tag="s")
                for c in range(CK):
                    nc.tensor.matmul(
                        qk_ps[:],
                        lhsT=wq_bf[c][:, m * 128:(m + 1) * 128],
                        rhs=xT_bf[c][:, qh * 512:(qh + 1) * 512],
                        start=(c == 0),
                        stop=(c == CK - 1),
                    )
                nc.vector.tensor_copy(qkT[m][:, qh * 512:(qh + 1) * 512], qk_ps[:])

        # v in natural layout [n, (head, d)] with a ones column appended
        # per head: v_sb[n] is [128, NH, D+1], [:, h, D] == 1.0.
        v_sb = [
            sb.tile([128, NH, D + 1], BF16, name=f"v_sb{n}", tag=f"v_sb{n}")
            for n in range(NQ)
        ]

        def emit_v(n):
            nc.gpsimd.memset(v_sb[n][:, :, D], 1.0)
            for half in range(2):
                v_ps = ps.tile([128, 384], F32, name=f"v_ps{n}_{half}", tag="s")
                for c in range(CK):
                    nc.tensor.matmul(
                        v_ps[:],
                        lhsT=xT_bf[c][:, n * 128:(n + 1) * 128],
                        rhs=wq_bf[c][:, 2 * C + half * 384:2 * C + (half + 1) * 384],
                        start=(c == 0),
                        stop=(c == CK - 1),
                    )
                nc.vector.tensor_copy(
                    v_sb[n][:, half * 6:(half + 1) * 6, 0:D],
                    v_ps[:].rearrange("p (h d) -> p h d", d=D),
                )

        # ---- attention ------------------------------------------------
        attn_bf = [
            sb.tile([128, N], BF16, name=f"attn_bf{p}", tag=f"attn_bf{p}")
            for p in range(6)
        ]

        def emit_head(h, filler=None):
            """S^T + exp + P@V for head h; `filler` emits extra PE work
            early in the stream (previous head's deferred normalize, next
            qkv chunk) so PE has exp-independent work while ScalarE runs."""
            q_tile = qkT[h // 2]
            k_tile = qkT[6 + h // 2]
            ro = (h % 2) * 64
            out_aug = acc.tile([D + 1, N], F32, name=f"oaug{h}", tag="acc")

            def emit_S(kc):
                st = ps.tile([128, N], F32, name=f"s{h}_{kc}", tag="s")
                for qh in range(2):
                    nc.tensor.matmul(
                        st[:, qh * 512:(qh + 1) * 512],
                        lhsT=k_tile[ro:ro + D, kc * 128:(kc + 1) * 128],
                        rhs=q_tile[ro:ro + D, qh * 512:(qh + 1) * 512],
                        start=True,
                        stop=True,
                    )
                pt = pp.tile([128, N], BF16, name=f"P{h}_{kc}", tag="P")
                nc.scalar.activation(pt[:], st[:], Exp, scale=SCALE)
                return pt

            def emit_v_mm(kc, pt):
                for qh in range(2):
                    nc.tensor.matmul(
                        out_aug[:, qh * 512:(qh + 1) * 512],
                        lhsT=v_sb[kc][:, h, :],
                        rhs=pt[:, qh * 512:(qh + 1) * 512],
                        start=(kc == 0),
                        stop=(kc == NQ - 1),
                    )

            # software pipeline: exp(kc) overlaps S(kc+1) and P@V(kc-1)
            pts = {0: emit_S(0), 1: emit_S(1)}
            if filler is not None:
                filler()
            for kc in range(NQ):
                emit_v_mm(kc, pts.pop(kc))
                if kc + 2 < NQ:
                    pts[kc + 2] = emit_S(kc + 2)
            return out_aug

        def emit_norm_pre(h, oa, direct=False):
            """DVE/GpSimd-only part: reciprocal chain first (it gates the
            final multiply), then stage the accumulator to SBUF to release
            its PSUM slot. For the last head (`direct`) the multiply reads
            the accumulator straight from PSUM instead — shortest tail."""
            if not direct:
                # staging copy FIRST: it releases the single-slot PSUM
                # accumulator, which gates the next head's P@V matmuls
                un = small.tile([D, N], F32, name=f"un{h}", tag="un")
                nc.vector.tensor_copy(un[:], oa[0:D, :])
            dn = small.tile([1, N], F32, name=f"dn{h}", tag="dn")
            nc.vector.tensor_copy(dn[:], oa[D:D + 1, :])
            rc = small.tile([1, N], F32, name=f"rc{h}", tag="rc")
            # reciprocal_approx_fast's uOp program only works from
            # partition 0 on hardware, hence the dn bounce copy above.
            nc.vector.reciprocal_approx_fast(rc[:], dn[:])
            rcb = small.tile([1, N], BF16, name=f"rcb{h}", tag="rcb")
            nc.vector.tensor_copy(rcb[:], rc[:])
            bcast = small.tile([64, N], BF16, name=f"bcast{h}", tag="bcast")
            nc.gpsimd.partition_broadcast(bcast[:], rcb[:])
            if direct:
                return oa, bcast
            return un, bcast

        def emit_norm_post(h, un, bcast):
            p, ro = h // 2, (h % 2) * 64
            nc.vector.tensor_mul(attn_bf[p][ro:ro + 64, :], un[0:D, :], bcast[:])

        emit_qk(0)
        emit_qk(6)
        for g in (4, 5):  # v weights: DMA landed by now, cast on DVE
            for c in range(CK):
                load_w(g, c)
        emit_v(0)

        # Remaining qkv work rides inside the attention stream as PE
        # filler during exp waits: head 0 carries the other v chunks
        # (needed from its own P@V loop onward) and head 1 the right-half
        # q/k weight casts; later heads each carry one q/k chunk, landing
        # one pair ahead of first use.
        QK_FILL = {1: (1, 7), 2: (2,), 3: (8,), 4: (3,), 5: (9,),
                   6: (4,), 7: (10,), 8: (5,), 9: (11,)}
        wp_bf = [
            sb.tile([128, C], BF16, name=f"wp_bf{c}", tag=f"wp_bf{c}")
            for c in range(CK)
        ]

        def load_wp(c):
            wp_st = stage.tile([128, C], F32, name=f"wp_st{c}", tag="stage")
            nc.sync.dma_start(out=wp_st[:], in_=wprojT[c * 128:(c + 1) * 128, :])
            nc.vector.tensor_copy(wp_bf[c][:], wp_st[:])

        pending = None
        for h in range(NH):
            fillers = []
            if h == 0:
                fillers.append(lambda: [emit_v(n) for n in range(1, NQ)])
            if h == 1:
                fillers.append(
                    lambda: [load_w(g, c) for g in (1, 3) for c in range(CK)]
                )
            for m in QK_FILL.get(h, ()):
                fillers.append(lambda m=m: emit_qk(m))
            if pending is not None:
                ph, un, bc = pending
                fillers.append(lambda ph=ph, un=un, bc=bc: emit_norm_post(ph, un, bc))

            def filler():
                for f in fillers:
                    f()

            oa = emit_head(h, filler=filler)
            pending = (h, *emit_norm_pre(h, oa))
        emit_norm_post(*pending)

        # ---- output projection ---------------------------------------
        acc.release()
        ps.release()
        yps = tc.alloc_tile_pool(name="yps", bufs=3, space="PSUM")

        # bias: load row, broadcast across partitions once (proj-only)
        bp_row = sb.tile([1, C], F32, name="bp_row", tag="bp_row")
        nc.sync.dma_start(out=bp_row[:], in_=bproj[None, :])
        bias_bc = sb.tile([128, C], F32, name="bias_bc", tag="bias_bc")
        nc.gpsimd.partition_broadcast(bias_bc[:], bp_row[:])

        for c in range(CK):
            load_wp(c)

        # Groups of 3 n-chunks, two sweeps each: the c<5 accumulations of
        # a whole group run first (PE work that doesn't need attn_bf[5],
        # absorbing head 11's normalize-chain latency), then the c=5
        # closers + bias-add + store.
        for grp in (range(0, 3), range(3, 6), range(6, NQ)):
            y_tiles = {}
            for n in grp:
                y_ps = yps.tile([128, C], F32, name=f"y_ps{n}", tag="y_ps")
                y_tiles[n] = y_ps
                for lo, hi in ((0, 512), (512, 768)):
                    for c in range(CK - 1):
                        nc.tensor.matmul(
                            y_ps[:, lo:hi],
                            lhsT=attn_bf[c][:, n * 128:(n + 1) * 128],
                            rhs=wp_bf[c][:, lo:hi],
                            start=(c == 0),
                            stop=False,
                        )
            for n in grp:
                y_ps = y_tiles[n]
                for lo, hi in ((0, 512), (512, 768)):
                    nc.tensor.matmul(
                        y_ps[:, lo:hi],
                        lhsT=attn_bf[CK - 1][:, n * 128:(n + 1) * 128],
                        rhs=wp_bf[CK - 1][:, lo:hi],
                        start=False,
                        stop=True,
                    )
                y_sb = stage.tile([128, C], F32, name=f"y_sb{n}", tag="y", bufs=2)
                nc.vector.tensor_add(y_sb[:], y_ps[:], bias_bc[:])
                nc.sync.dma_start(out=out[n * 128:(n + 1) * 128, :], in_=y_sb[:])
        yps.release()


def build_graph():
    nc = bacc.Bacc("TRN2", target_bir_lowering=False, debug=False)
    xT = nc.declare_dram_parameter("xT", [C, N], F32, isOutput=False)
    wqkvT = nc.declare_dram_parameter("wqkvT", [C, 3 * C], F32, isOutput=False)
    wprojT = nc.declare_dram_parameter("wprojT", [C, C], F32, isOutput=False)
    bproj = nc.declare_dram_parameter("bproj", [C], F32, isOutput=False)
    out = nc.declare_dram_parameter("out", [N, C], F32, isOutput=True)
    with tile.TileContext(nc) as tc:
        _emit(tc, xT.ap(), wqkvT.ap(), wprojT.ap(), bproj.ap(), out.ap())
    nc.compile()
    return nc


_GRAPH = None


def _get_graph():
    global _GRAPH
    if _GRAPH is None:
        _GRAPH = build_graph()
    return _GRAPH


def make_in_maps(x, W_qkv, W_proj, b_proj):
    x = np.ascontiguousarray(np.asarray(x, dtype=np.float32))
    wqkvT = np.ascontiguousarray(np.asarray(W_qkv, dtype=np.float32).T)
    wprojT = np.ascontiguousarray(np.asarray(W_proj, dtype=np.float32).T)
    bp = np.ascontiguousarray(np.asarray(b_proj, dtype=np.float32))
    xT_all = np.ascontiguousarray(x.transpose(0, 2, 1))
    return [
        {"xT": xT_all[i], "wqkvT": wqkvT, "wprojT": wprojT, "bproj": bp}
        for i in range(B)
    ]


def run(x, W_qkv, W_proj, b_proj, trace=False):
    nc = _get_graph()
    in_maps = make_in_maps(x, W_qkv, W_proj, b_proj)
    res = run_bass_kernel_spmd(nc, in_maps, core_ids=list(range(B)), trace=trace)
    out = np.stack([res.results[i]["out"] for i in range(B)], axis=0)
    return out.astype(np.float32, copy=False), res


def kernel(x, W_qkv, W_proj, b_proj, H=None, W=None):
    out, _ = run(x, W_qkv, W_proj, b_proj)
    return out



# revision 6
# speedup vs baseline: 1.0838x; 1.0838x over previous
"""Multi-head attention (B=8, N=1024, C=768, 12 heads) on 8 TRN2 NeuronCores.

Sharding: data-parallel over batch — batch element b runs on core b, weights
replicated, zero collectives.

Per-core kernel (all matmuls bf16 on the TensorEngine):
  - Host pre-transposes AND pre-converts x, W_qkv, W_proj to bf16: every
    contraction has its reduction axis on SBUF partitions, DMA bytes are
    halved, and no on-device dtype casts are needed at all.
  - scores are computed TRANSPOSED per head (S^T[k,q], lhsT=k^T-block,
    rhs=q^T-block) so exp's output P^T feeds P@V directly as the moving
    operand. The 1/sqrt(d) scale rides on the exp's affine pre-scale.
  - HEAD-PAIR PACKING: head 2p lives on SBUF partitions 0:64 of qkT chunk
    p, head 2p+1 on 64:128. Their K=64 score matmuls are emitted
    back-to-back with auto-derived tile_position (0,0)/(64,0): the PE runs
    them CONCURRENTLY in disjoint row-group halves of the array, doubling
    score throughput vs. serial K=64 matmuls.
  - each step's paired scores land in one [128,1024] PSUM tile (head 2p in
    cols 0:512, head 2p+1 in 512:1024) -> a single FD=1024 exp per step.
  - softmax denominators come free: v is stored with a ones-column
    appended per head (lhsT [128,65]); row 64 of the P@V accumulator is
    sum_k exp(S).
  - PSUM (8 banks) budget: S double-buffer 4 + one P@V accumulator 2 +
    qkv-filler 2. The single accumulator forces head 2p+1's P@V to lag
    one phase behind its exp (P^T tiles buffer in SBUF meanwhile): each
    phase runs head 2p-1's P@V dense in its first half, hands the
    accumulator off, then runs head 2p's P@V in the second half.
  - qkv projection chunks and v emission ride inside the attention stream
    as PE filler during exp waits; weight DMA is priority-ordered so the
    first pair's q/k columns land first.
  - proj: y = attn @ W_proj^T + b_proj, c<5 accumulation sweeps first so
    the last head's normalize latency is hidden.
"""

from contextlib import ExitStack

import ml_dtypes
import numpy as np

import concourse.mybir as mybir
import concourse.tile as tile
from concourse import bacc
from concourse.bass_utils import run_bass_kernel_spmd

B, N, C = 8, 1024, 768
NH, D = 12, 64
CK = C // 128  # 6 contraction chunks of 128
NQ = N // 128  # 8 position chunks of 128
NPAIR = NH // 2
SCALE = D ** -0.5
F32 = mybir.dt.float32
BF16 = mybir.dt.bfloat16
Exp = mybir.ActivationFunctionType.Exp


def _emit(tc, xT, wqkvT, wprojT, bproj, out):
    nc = tc.nc
    with ExitStack() as ctx:
        sb = ctx.enter_context(tc.tile_pool(name="sb", bufs=1))
        pp = ctx.enter_context(tc.tile_pool(name="pp", bufs=22))
        small = ctx.enter_context(tc.tile_pool(name="small", bufs=2))
        stage = ctx.enter_context(tc.tile_pool(name="stage", bufs=3))
        # PSUM: acc 1x[65,1024] (2 banks) + fill 2x[128,512] (2 banks)
        # + spool 2x[128,1024] (4 banks) = 8 banks exactly. acc first so
        # spool+fill can release (LIFO) for the projection's yps pool.
        acc = tc.alloc_tile_pool(name="acc", bufs=1, space="PSUM")
        fill = tc.alloc_tile_pool(name="fill", bufs=2, space="PSUM")
        spool = tc.alloc_tile_pool(name="spool", bufs=2, space="PSUM")

        # ---- PE warm-up ----------------------------------------------
        warm_in = sb.tile([128, 512], BF16, name="warm_in", tag="warm_in")
        nc.gpsimd.memset(warm_in[:], 1.0)
        warm_ps = fill.tile([128, 512], F32, name="warm_ps", tag="fill")
        for i in range(16):
            nc.tensor.matmul(
                warm_ps[:],
                lhsT=warm_in[:, 0:128],
                rhs=warm_in[:],
                start=(i == 0),
                stop=(i == 15),
            )

        # ---- input DMA, priority-ordered -----------------------------
        xT_bf = [
            sb.tile([128, N], BF16, name=f"xT{c}", tag=f"xT{c}") for c in range(CK)
        ]
        wq_bf = [
            sb.tile([128, 3 * C], BF16, name=f"wq{c}", tag=f"wq{c}")
            for c in range(CK)
        ]
        for c in range(CK):
            nc.sync.dma_start(out=xT_bf[c][:], in_=xT[c * 128:(c + 1) * 128, :])

        def dma_w(m, c):
            nc.sync.dma_start(
                out=wq_bf[c][:, m * 128:(m + 1) * 128],
                in_=wqkvT[c * 128:(c + 1) * 128, m * 128:(m + 1) * 128],
            )

        for m in (0, 6):  # pair-0 q/k columns first
            for c in range(CK):
                dma_w(m, c)
        for c in range(CK):  # v columns next (v emission starts in lead-in)
            nc.sync.dma_start(
                out=wq_bf[c][:, 2 * C:3 * C],
                in_=wqkvT[c * 128:(c + 1) * 128, 2 * C:3 * C],
            )
        for m in (1, 7, 2, 8, 3, 9, 4, 10, 5, 11):  # phase order
            for c in range(CK):
                dma_w(m, c)
        wp_bf = [
            sb.tile([128, C], BF16, name=f"wp{c}", tag=f"wp{c}") for c in range(CK)
        ]
        for c in range(CK):
            nc.sync.dma_start(out=wp_bf[c][:], in_=wprojT[c * 128:(c + 1) * 128, :])
        bp_row = sb.tile([1, C], F32, name="bp_row", tag="bp_row")
        nc.sync.dma_start(out=bp_row[:], in_=bproj[None, :])
        bias_bc = sb.tile([128, C], F32, name="bias_bc", tag="bias_bc")
        nc.gpsimd.partition_broadcast(bias_bc[:], bp_row[:])

        # ---- qkv emission helpers ------------------------------------
        qkT = [
            sb.tile([128, N], BF16, name=f"qkT{m}", tag=f"qkT{m}")
            for m in range(12)
        ]

        def emit_qk_half(m, qh):
            ps = fill.tile([128, 512], F32, name=f"qk{m}_{qh}", tag="fill")
            for c in range(CK):
                nc.tensor.matmul(
                    ps[:],
                    lhsT=wq_bf[c][:, m * 128:(m + 1) * 128],
                    rhs=xT_bf[c][:, qh * 512:(qh + 1) * 512],
                    start=(c == 0),
                    stop=(c == CK - 1),
                )
            nc.vector.tensor_copy(qkT[m][:, qh * 512:(qh + 1) * 512], ps[:])

        v_sb = [
            sb.tile([128, NH, D + 1], BF16, name=f"v{n}", tag=f"v{n}")
            for n in range(NQ)
        ]

        def emit_v(n):
            nc.gpsimd.memset(v_sb[n][:, :, D], 1.0)
            for half in range(2):
                ps = fill.tile([128, 512], F32, name=f"v{n}_{half}", tag="fill")
                for c in range(CK):
                    nc.tensor.matmul(
                        ps[:, 0:384],
                        lhsT=xT_bf[c][:, n * 128:(n + 1) * 128],
                        rhs=wq_bf[c][:, 2 * C + half * 384:2 * C + (half + 1) * 384],
                        start=(c == 0),
                        stop=(c == CK - 1),
                    )
                nc.vector.tensor_copy(
                    v_sb[n][:, half * 6:(half + 1) * 6, 0:D],
                    ps[:, 0:384].rearrange("p (h d) -> p h d", d=D),
                )

        # ---- attention building blocks -------------------------------
        attn_bf = [
            sb.tile([128, N], BF16, name=f"attn{p}", tag=f"attn{p}")
            for p in range(NPAIR)
        ]

        def emit_S(p, s):
            """Paired scores for step s=(kc,qh) of pair p: head 2p on PE
            rows 0:64 -> cols 0:512, head 2p+1 on rows 64:128 -> cols
            512:1024 (concurrent row-group tiles), one FD=1024 exp."""
            kc, qh = s // 2, s % 2
            q, k = qkT[p], qkT[6 + p]
            st = spool.tile([128, 1024], F32, name=f"s{p}_{s}", tag="s")
            nc.tensor.matmul(
                st[:, 0:512],
                lhsT=k[0:64, kc * 128:(kc + 1) * 128],
                rhs=q[0:64, qh * 512:(qh + 1) * 512],
                start=True,
                stop=True,
            )
            nc.tensor.matmul(
                st[:, 512:1024],
                lhsT=k[64:128, kc * 128:(kc + 1) * 128],
                rhs=q[64:128, qh * 512:(qh + 1) * 512],
                start=True,
                stop=True,
            )
            pt = pp.tile([128, 1024], BF16, name=f"P{p}_{s}", tag="P")
            nc.scalar.activation(pt[:], st[:], Exp, scale=SCALE)
            return pt

        def emit_pv(oa, h, pt, s):
            kc, qh = s // 2, s % 2
            nc.tensor.matmul(
                oa[:, qh * 512:(qh + 1) * 512],
                lhsT=v_sb[kc][:, h, :],
                rhs=pt[:, (h % 2) * 512:(h % 2) * 512 + 512],
                start=(kc == 0),
                stop=(kc == NQ - 1),
            )

        def norm_pre(h, oa, direct=False):
            """Reciprocal chain + accumulator staging; the staging copy
            releases the single-slot PSUM accumulator. For the last head
            (`direct`) the multiply reads PSUM directly instead."""
            if not direct:
                un = small.tile([D, N], F32, name=f"un{h}", tag="un")
                nc.vector.tensor_copy(un[:], oa[0:D, :])
            dn = small.tile([1, N], F32, name=f"dn{h}", tag="dn")
            nc.vector.tensor_copy(dn[:], oa[D:D + 1, :])
            rc = small.tile([1, N], F32, name=f"rc{h}", tag="rc")
            nc.vector.reciprocal_approx_fast(rc[:], dn[:])
            rcb = small.tile([1, N], BF16, name=f"rcb{h}", tag="rcb")
            nc.vector.tensor_copy(rcb[:], rc[:])
            bcast = small.tile([64, N], BF16, name=f"bcast{h}", tag="bcast")
            nc.gpsimd.partition_broadcast(bcast[:], rcb[:])
            if direct:
                return oa, bcast
            return un, bcast

        def norm_post(h, un, bcast):
            p, ro = h // 2, (h % 2) * 64
            nc.vector.tensor_mul(attn_bf[p][ro:ro + 64, :], un[0:D, :], bcast[:])

        # ---- lead-in: pair-0 q/k + first v chunks --------------------
        for qh in range(2):
            emit_qk_half(0, qh)
        for qh in range(2):
            emit_qk_half(6, qh)
        for n in range(4):
            emit_v(n)

        # ---- attention phases ----------------------------------------
        # Phase p: 16 S/exp steps for pair p. First half also drains head
        # 2p-1's P@V (its P tiles buffered from phase p-1, 2 MMs/step),
        # then the accumulator hands off and head 2p's P@V runs in the
        # second half (2 MMs/step, consuming this phase's P tiles).
        # Fillers: remaining v chunks + pair-1 q/k (phase 0), pair p+1's
        # q/k chunks (later phases), pending normalize multiplies.
        pts_prev = None
        norm_pending = []  # (head, un, bcast)
        oa_prev = None

        for p in range(NPAIR):
            h0, h1_prev = 2 * p, 2 * p - 1
            pts = []
            oa0 = None
            if p == 0:
                fill_work = [lambda n=n: emit_v(n) for n in range(4, NQ)]
                for m in (1, 7):
                    fill_work += [
                        lambda m=m, qh=qh: emit_qk_half(m, qh) for qh in range(2)
                    ]
                fill_at = {1: 0, 3: 1, 5: 2, 7: 3, 9: 4, 11: 5, 13: 6, 15: 7}
            elif p + 1 < NPAIR:
                fill_work = []
                for m in (p + 1, 7 + p):
                    fill_work += [
                        lambda m=m, qh=qh: emit_qk_half(m, qh) for qh in range(2)
                    ]
                fill_at = {1: 0, 5: 1, 9: 2, 13: 3}
            else:
                fill_work, fill_at = [], {}

            for s in range(16):
                pts.append(emit_S(p, s))
                if s < 8:
                    if pts_prev is not None:
                        emit_pv(oa_prev, h1_prev, pts_prev[2 * s], 2 * s)
                        emit_pv(oa_prev, h1_prev, pts_prev[2 * s + 1], 2 * s + 1)
                else:
                    if s == 8:
                        if pts_prev is not None:
                            norm_pending.append(
                                (h1_prev, *norm_pre(h1_prev, oa_prev))
                            )
                        oa0 = acc.tile([D + 1, N], F32, name=f"oa{h0}", tag="acc")
                    emit_pv(oa0, h0, pts[2 * (s - 8)], 2 * (s - 8))
                    emit_pv(oa0, h0, pts[2 * (s - 8) + 1], 2 * (s - 8) + 1)
                if s in fill_at:
                    fill_work[fill_at[s]]()
                if s in (3, 11) and norm_pending:
                    norm_post(*norm_pending.pop(0))
            norm_pending.append((h0, *norm_pre(h0, oa0)))
            oa_prev = acc.tile([D + 1, N], F32, name=f"oa{2 * p + 1}", tag="acc")
            pts_prev, pts = pts, None

        # ---- drain: last head's P@V + remaining normalizes -----------
        h_last = NH - 1
        for s in range(16):
            emit_pv(oa_prev, h_last, pts_prev[s], s)
        while norm_pending:
            norm_post(*norm_pending.pop(0))
        norm_pending.append((h_last, *norm_pre(h_last, oa_prev, direct=True)))

        # ---- output projection ---------------------------------------
        spool.release()
        fill.release()
        yps = tc.alloc_tile_pool(name="yps", bufs=3, space="PSUM")

        # Groups of 3 n-chunks, two sweeps each: c<5 accumulations first
        # (PE work that doesn't need attn_bf[5]'s second half, absorbing
        # the last head's normalize latency), then c=5 closers + bias-add
        # + store.
        done_last = False
        for grp in (range(0, 3), range(3, 6), range(6, NQ)):
            y_tiles = {}
            for n in grp:
                y_ps = yps.tile([128, C], F32, name=f"y_ps{n}", tag="y_ps")
                y_tiles[n] = y_ps
                for lo, hi in ((0, 512), (512, 768)):
                    for c in range(CK - 1):
                        nc.tensor.matmul(
                            y_ps[:, lo:hi],
                            lhsT=attn_bf[c][:, n * 128:(n + 1) * 128],
                            rhs=wp_bf[c][:, lo:hi],
                            start=(c == 0),
                            stop=False,
                        )
            if not done_last:
                # last head's normalize multiply, reading PSUM directly
                norm_post(*norm_pending.pop(0))
                done_last = True
            for n in grp:
                y_ps = y_tiles[n]
                for lo, hi in ((0, 512), (512, 768)):
                    nc.tensor.matmul(
                        y_ps[:, lo:hi],
                        lhsT=attn_bf[CK - 1][:, n * 128:(n + 1) * 128],
                        rhs=wp_bf[CK - 1][:, lo:hi],
                        start=False,
                        stop=True,
                    )
                y_sb = stage.tile([128, C], F32, name=f"y_sb{n}", tag="y")
                nc.vector.tensor_add(y_sb[:], y_ps[:], bias_bc[:])
                nc.sync.dma_start(out=out[n * 128:(n + 1) * 128, :], in_=y_sb[:])
        yps.release()
        acc.release()


def build_graph():
    nc = bacc.Bacc("TRN2", target_bir_lowering=False, debug=False)
    xT = nc.declare_dram_parameter("xT", [C, N], BF16, isOutput=False)
    wqkvT = nc.declare_dram_parameter("wqkvT", [C, 3 * C], BF16, isOutput=False)
    wprojT = nc.declare_dram_parameter("wprojT", [C, C], BF16, isOutput=False)
    bproj = nc.declare_dram_parameter("bproj", [C], F32, isOutput=False)
    out = nc.declare_dram_parameter("out", [N, C], F32, isOutput=True)
    with tile.TileContext(nc) as tc:
        _emit(tc, xT.ap(), wqkvT.ap(), wprojT.ap(), bproj.ap(), out.ap())
    nc.compile()
    return nc


_GRAPH = None


def _get_graph():
    global _GRAPH
    if _GRAPH is None:
        _GRAPH = build_graph()
    return _GRAPH


def make_in_maps(x, W_qkv, W_proj, b_proj):
    bf = ml_dtypes.bfloat16
    x = np.asarray(x, dtype=np.float32)
    xT_all = np.ascontiguousarray(x.transpose(0, 2, 1).astype(bf))
    wqkvT = np.ascontiguousarray(np.asarray(W_qkv, dtype=np.float32).T.astype(bf))
    wprojT = np.ascontiguousarray(np.asarray(W_proj, dtype=np.float32).T.astype(bf))
    bp = np.ascontiguousarray(np.asarray(b_proj, dtype=np.float32))
    return [
        {"xT": xT_all[i], "wqkvT": wqkvT, "wprojT": wprojT, "bproj": bp}
        for i in range(B)
    ]


def run(x, W_qkv, W_proj, b_proj, trace=False):
    nc = _get_graph()
    in_maps = make_in_maps(x, W_qkv, W_proj, b_proj)
    res = run_bass_kernel_spmd(nc, in_maps, core_ids=list(range(B)), trace=trace)
    out = np.stack([res.results[i]["out"] for i in range(B)], axis=0)
    return out.astype(np.float32, copy=False), res


def kernel(x, W_qkv, W_proj, b_proj, H=None, W=None):
    out, _ = run(x, W_qkv, W_proj, b_proj)
    return out


# revision 13
# speedup vs baseline: 1.1330x; 1.0454x over previous
"""Multi-head attention (B=8, N=1024, C=768, 12 heads) on 8 TRN2 NeuronCores.

Sharding: data-parallel over batch — batch element b runs on core b, weights
replicated, zero collectives.

Per-core kernel (all matmuls bf16 on the TensorEngine):
  - Host pre-transposes AND pre-converts x, W_qkv, W_proj to bf16: every
    contraction has its reduction axis on SBUF partitions, DMA bytes are
    halved, and no on-device dtype casts are needed at all.
  - scores are computed TRANSPOSED per head (S^T[k,q], lhsT=k^T-block,
    rhs=q^T-block) so exp's output P^T feeds P@V directly as the moving
    operand. The 1/sqrt(d) scale rides on the exp's affine pre-scale.
  - HEAD-PAIR PACKING: head 2p lives on SBUF partitions 0:64 of qkT chunk
    p, head 2p+1 on 64:128. Their K=64 score matmuls are emitted
    back-to-back with auto-derived tile_position (0,0)/(64,0): the PE runs
    them CONCURRENTLY in disjoint row-group halves of the array, doubling
    score throughput vs. serial K=64 matmuls.
  - each step's paired scores land in one [128,1024] PSUM tile (head 2p in
    cols 0:512, head 2p+1 in 512:1024) -> a single FD=1024 exp per step.
  - softmax denominators come free: v is stored with a ones-column
    appended per head (lhsT [128,65]); row 64 of the P@V accumulator is
    sum_k exp(S).
  - PSUM (8 banks) budget: S double-buffer 4 + one P@V accumulator 2 +
    qkv-filler 2. The single accumulator forces head 2p+1's P@V to lag
    one phase behind its exp (P^T tiles buffer in SBUF meanwhile): each
    phase runs head 2p-1's P@V dense in its first half, hands the
    accumulator off, then runs head 2p's P@V in the second half.
  - qkv projection chunks and v emission ride inside the attention stream
    as PE filler during exp waits; weight DMA is priority-ordered so the
    first pair's q/k columns land first.
  - proj: y = attn @ W_proj^T + b_proj, c<5 accumulation sweeps first so
    the last head's normalize latency is hidden.
"""

from contextlib import ExitStack

import ml_dtypes
import numpy as np

import concourse.mybir as mybir
import concourse.tile as tile
from concourse import bacc
from concourse.bass_utils import run_bass_kernel_spmd

B, N, C = 8, 1024, 768
NH, D = 12, 64
CK = C // 128  # 6 contraction chunks of 128
NQ = N // 128  # 8 position chunks of 128
NPAIR = NH // 2
SCALE = D ** -0.5
F32 = mybir.dt.float32
BF16 = mybir.dt.bfloat16
Exp = mybir.ActivationFunctionType.Exp


def _emit(tc, xT, wqkvT, wprojT, bproj, out):
    nc = tc.nc
    with ExitStack() as ctx:
        sb = ctx.enter_context(tc.tile_pool(name="sb", bufs=1))
        pp = ctx.enter_context(tc.tile_pool(name="pp", bufs=22))
        small = ctx.enter_context(tc.tile_pool(name="small", bufs=2))
        stage = ctx.enter_context(tc.tile_pool(name="stage", bufs=3))
        # PSUM: acc 1x[65,1024] (2 banks) + fill 2x[128,512] (2 banks)
        # + spool 2x[128,1024] (4 banks) = 8 banks exactly. acc first so
        # spool+fill can release (LIFO) for the projection's yps pool.
        acc = tc.alloc_tile_pool(name="acc", bufs=1, space="PSUM")
        fill = tc.alloc_tile_pool(name="fill", bufs=2, space="PSUM")
        spool = tc.alloc_tile_pool(name="spool", bufs=2, space="PSUM")

        # ---- PE warm-up ----------------------------------------------
        warm_in = sb.tile([128, 512], BF16, name="warm_in", tag="warm_in")
        nc.gpsimd.memset(warm_in[:], 1.0)
        warm_ps = fill.tile([128, 512], F32, name="warm_ps", tag="fill")
        for i in range(16):
            nc.tensor.matmul(
                warm_ps[:],
                lhsT=warm_in[:, 0:128],
                rhs=warm_in[:],
                start=(i == 0),
                stop=(i == 15),
            )

        # ---- input DMA, priority-ordered -----------------------------
        xT_bf = [
            sb.tile([128, N], BF16, name=f"xT{c}", tag=f"xT{c}") for c in range(CK)
        ]
        wq_bf = [
            sb.tile([128, 3 * C], BF16, name=f"wq{c}", tag=f"wq{c}")
            for c in range(CK)
        ]
        # W_qkv is host-permuted into first-use order (see make_in_maps):
        # cols [m0 | m6 | v(768) | m1 | m7 | m2 | m8 | m3 | m9 | m4 | m10
        # | m5 | m11], so the DMA splits into three priority blocks per
        # c-chunk with >=512B-per-partition descriptors.
        for c in range(CK):
            nc.sync.dma_start(out=xT_bf[c][:], in_=xT[c * 128:(c + 1) * 128, :])
        for lo, hi in ((0, 256), (256, 1024), (1024, 2304)):
            for c in range(CK):
                nc.sync.dma_start(
                    out=wq_bf[c][:, lo:hi],
                    in_=wqkvT[c * 128:(c + 1) * 128, lo:hi],
                )
        wp_bf = [
            sb.tile([128, C], BF16, name=f"wp{c}", tag=f"wp{c}") for c in range(CK)
        ]
        for c in range(CK):
            nc.sync.dma_start(out=wp_bf[c][:], in_=wprojT[c * 128:(c + 1) * 128, :])
        bp_row = sb.tile([1, C], F32, name="bp_row", tag="bp_row")
        nc.sync.dma_start(out=bp_row[:], in_=bproj[None, :])
        bias_bc = sb.tile([128, C], F32, name="bias_bc", tag="bias_bc")
        nc.gpsimd.partition_broadcast(bias_bc[:], bp_row[:])

        # ---- qkv emission helpers ------------------------------------
        qkT = [
            sb.tile([128, N], BF16, name=f"qkT{m}", tag=f"qkT{m}")
            for m in range(12)
        ]

        # column offset of q/k chunk m in the host-permuted W_qkv layout
        W_COL = {0: 0, 6: 128, 1: 1024, 7: 1152, 2: 1280, 8: 1408,
                 3: 1536, 9: 1664, 4: 1792, 10: 1920, 5: 2048, 11: 2176}
        W_VCOL = 256  # v block: cols 256..1024

        def emit_qk_half(m, qh):
            ps = fill.tile([128, 512], F32, name=f"qk{m}_{qh}", tag="fill")
            wc = W_COL[m]
            for c in range(CK):
                nc.tensor.matmul(
                    ps[:],
                    lhsT=wq_bf[c][:, wc:wc + 128],
                    rhs=xT_bf[c][:, qh * 512:(qh + 1) * 512],
                    start=(c == 0),
                    stop=(c == CK - 1),
                )
            nc.vector.tensor_copy(qkT[m][:, qh * 512:(qh + 1) * 512], ps[:])

        v_sb = [
            sb.tile([128, NH, D + 1], BF16, name=f"v{n}", tag=f"v{n}")
            for n in range(NQ)
        ]

        def emit_v(n):
            nc.gpsimd.memset(v_sb[n][:, :, D], 1.0)
            for half in range(2):
                ps = fill.tile([128, 512], F32, name=f"v{n}_{half}", tag="fill")
                for c in range(CK):
                    nc.tensor.matmul(
                        ps[:, 0:384],
                        lhsT=xT_bf[c][:, n * 128:(n + 1) * 128],
                        rhs=wq_bf[c][:, W_VCOL + half * 384:W_VCOL + (half + 1) * 384],
                        start=(c == 0),
                        stop=(c == CK - 1),
                    )
                nc.vector.tensor_copy(
                    v_sb[n][:, half * 6:(half + 1) * 6, 0:D],
                    ps[:, 0:384].rearrange("p (h d) -> p h d", d=D),
                )

        # ---- attention building blocks -------------------------------
        attn_bf = [
            sb.tile([128, N], BF16, name=f"attn{p}", tag=f"attn{p}")
            for p in range(NPAIR)
        ]

        def emit_S(p, s):
            """Paired scores for step s=(kc,qh) of pair p: head 2p on PE
            rows 0:64 -> cols 0:512, head 2p+1 on rows 64:128 -> cols
            512:1024 (concurrent row-group tiles), one FD=1024 exp."""
            kc, qh = s // 2, s % 2
            q, k = qkT[p], qkT[6 + p]
            st = spool.tile([128, 1024], F32, name=f"s{p}_{s}", tag="s")
            nc.tensor.matmul(
                st[:, 0:512],
                lhsT=k[0:64, kc * 128:(kc + 1) * 128],
                rhs=q[0:64, qh * 512:(qh + 1) * 512],
                start=True,
                stop=True,
            )
            nc.tensor.matmul(
                st[:, 512:1024],
                lhsT=k[64:128, kc * 128:(kc + 1) * 128],
                rhs=q[64:128, qh * 512:(qh + 1) * 512],
                start=True,
                stop=True,
            )
            pt = pp.tile([128, 1024], BF16, name=f"P{p}_{s}", tag="P")
            nc.scalar.activation(pt[:], st[:], Exp, scale=SCALE)
            return pt

        def emit_pv(oa, h, pt, s):
            kc, qh = s // 2, s % 2
            nc.tensor.matmul(
                oa[:, qh * 512:(qh + 1) * 512],
                lhsT=v_sb[kc][:, h, :],
                rhs=pt[:, (h % 2) * 512:(h % 2) * 512 + 512],
                start=(kc == 0),
                stop=(kc == NQ - 1),
            )

        def norm_pre(h, oa, direct=False):
            """Reciprocal chain + accumulator staging; the staging copy
            releases the single-slot PSUM accumulator. For the last head
            (`direct`) the multiply reads PSUM directly instead."""
            if not direct:
                un = small.tile([D, N], F32, name=f"un{h}", tag="un")
                nc.vector.tensor_copy(un[:], oa[0:D, :])
            dn = small.tile([1, N], F32, name=f"dn{h}", tag="dn")
            nc.vector.tensor_copy(dn[:], oa[D:D + 1, :])
            rc = small.tile([1, N], F32, name=f"rc{h}", tag="rc")
            nc.vector.reciprocal_approx_fast(rc[:], dn[:])
            rcb = small.tile([1, N], BF16, name=f"rcb{h}", tag="rcb")
            nc.vector.tensor_copy(rcb[:], rc[:])
            bcast = small.tile([64, N], BF16, name=f"bcast{h}", tag="bcast")
            nc.gpsimd.partition_broadcast(bcast[:], rcb[:])
            if direct:
                return oa, bcast
            return un, bcast

        def norm_post(h, un, bcast):
            p, ro = h // 2, (h % 2) * 64
            nc.vector.tensor_mul(attn_bf[p][ro:ro + 64, :], un[0:D, :], bcast[:])

        # ---- lead-in: pair-0 q/k + first v chunks --------------------
        for qh in range(2):
            emit_qk_half(0, qh)
        for qh in range(2):
            emit_qk_half(6, qh)
        for n in range(4):
            emit_v(n)

        # ---- attention phases ----------------------------------------
        # Phase p: 16 S/exp steps for pair p. First half also drains head
        # 2p-1's P@V (its P tiles buffered from phase p-1, 2 MMs/step),
        # then the accumulator hands off and head 2p's P@V runs in the
        # second half (2 MMs/step, consuming this phase's P tiles).
        # Fillers: remaining v chunks + pair-1 q/k (phase 0), pair p+1's
        # q/k chunks (later phases), pending normalize multiplies.
        pts_prev = None
        norm_pending = []  # (head, un, bcast)
        oa_prev = None

        for p in range(NPAIR):
            h0, h1_prev = 2 * p, 2 * p - 1
            pts = []
            oa0 = None
            if p == 0:
                fill_work = [lambda n=n: emit_v(n) for n in range(4, NQ)]
                for m in (1, 7):
                    fill_work += [
                        lambda m=m, qh=qh: emit_qk_half(m, qh) for qh in range(2)
                    ]
                fill_at = {1: 0, 3: 1, 5: 2, 7: 3, 9: 4, 11: 5, 13: 6, 15: 7}
            elif p + 1 < NPAIR:
                fill_work = []
                for m in (p + 1, 7 + p):
                    fill_work += [
                        lambda m=m, qh=qh: emit_qk_half(m, qh) for qh in range(2)
                    ]
                fill_at = {1: 0, 5: 1, 9: 2, 13: 3}
            else:
                fill_work, fill_at = [], {}

            for s in range(16):
                pts.append(emit_S(p, s))
                if s < 8:
                    if pts_prev is not None:
                        emit_pv(oa_prev, h1_prev, pts_prev[2 * s], 2 * s)
                        emit_pv(oa_prev, h1_prev, pts_prev[2 * s + 1], 2 * s + 1)
                else:
                    if s == 8:
                        if pts_prev is not None:
                            norm_pending.append(
                                (h1_prev, *norm_pre(h1_prev, oa_prev))
                            )
                        oa0 = acc.tile([D + 1, N], F32, name=f"oa{h0}", tag="acc")
                    emit_pv(oa0, h0, pts[2 * (s - 8)], 2 * (s - 8))
                    emit_pv(oa0, h0, pts[2 * (s - 8) + 1], 2 * (s - 8) + 1)
                if s in fill_at:
                    fill_work[fill_at[s]]()
                if s in (3, 11) and norm_pending:
                    norm_post(*norm_pending.pop(0))
            norm_pending.append((h0, *norm_pre(h0, oa0)))
            oa_prev = acc.tile([D + 1, N], F32, name=f"oa{2 * p + 1}", tag="acc")
            pts_prev, pts = pts, None

        # ---- drain: last head's P@V + remaining normalizes -----------
        h_last = NH - 1
        for s in range(16):
            emit_pv(oa_prev, h_last, pts_prev[s], s)
        # The last head's reciprocal chain gates the c=5 proj closers:
        # emit it FIRST on the DVE queue (its multiply reads the PSUM
        # accumulator directly), earlier heads' multiplies queue after.
        last_norm = (h_last, *norm_pre(h_last, oa_prev, direct=True))
        while norm_pending:
            norm_post(*norm_pending.pop(0))

        # ---- output projection ---------------------------------------
        spool.release()
        fill.release()
        yps = tc.alloc_tile_pool(name="yps", bufs=3, space="PSUM")

        # All c<5 accumulation sweeps for the first four n-chunks run
        # while the last head's normalize chain drains; c=5 closers +
        # bias-add + store follow, then the second four n-chunks.
        y_tiles = {}

        def proj_sweep(n):
            y_ps = yps.tile([128, C], F32, name=f"y_ps{n}", tag="y_ps")
            y_tiles[n] = y_ps
            for lo, hi in ((0, 512), (512, 768)):
                for c in range(CK - 1):
                    nc.tensor.matmul(
                        y_ps[:, lo:hi],
                        lhsT=attn_bf[c][:, n * 128:(n + 1) * 128],
                        rhs=wp_bf[c][:, lo:hi],
                        start=(c == 0),
                        stop=False,
                    )

        def proj_close(n):
            y_ps = y_tiles.pop(n)
            for lo, hi in ((0, 512), (512, 768)):
                nc.tensor.matmul(
                    y_ps[:, lo:hi],
                    lhsT=attn_bf[CK - 1][:, n * 128:(n + 1) * 128],
                    rhs=wp_bf[CK - 1][:, lo:hi],
                    start=False,
                    stop=True,
                )
            y_sb = stage.tile([128, C], F32, name=f"y_sb{n}", tag="y")
            nc.vector.tensor_add(y_sb[:], y_ps[:], bias_bc[:])
            nc.sync.dma_start(out=out[n * 128:(n + 1) * 128, :], in_=y_sb[:])

        for n in range(3):
            proj_sweep(n)
        norm_post(*last_norm)  # attn_bf[5] second half now complete
        for n in range(3):
            proj_close(n)
        for n in range(3, NQ):
            proj_sweep(n)
            proj_close(n)
        yps.release()
        acc.release()


def build_graph():
    nc = bacc.Bacc("TRN2", target_bir_lowering=False, debug=False)
    xT = nc.declare_dram_parameter("xT", [C, N], BF16, isOutput=False)
    wqkvT = nc.declare_dram_parameter("wqkvT", [C, 3 * C], BF16, isOutput=False)
    wprojT = nc.declare_dram_parameter("wprojT", [C, C], BF16, isOutput=False)
    bproj = nc.declare_dram_parameter("bproj", [C], F32, isOutput=False)
    out = nc.declare_dram_parameter("out", [N, C], F32, isOutput=True)
    with tile.TileContext(nc) as tc:
        _emit(tc, xT.ap(), wqkvT.ap(), wprojT.ap(), bproj.ap(), out.ap())
    nc.compile()
    return nc


_GRAPH = None


def _get_graph():
    global _GRAPH
    if _GRAPH is None:
        _GRAPH = build_graph()
    return _GRAPH


def make_in_maps(x, W_qkv, W_proj, b_proj):
    bf = ml_dtypes.bfloat16
    x = np.asarray(x, dtype=np.float32)
    xT_all = np.ascontiguousarray(x.transpose(0, 2, 1).astype(bf))
    wqkvT = np.asarray(W_qkv, dtype=np.float32).T  # [C, 3C]
    # first-use column permutation: m0 | m6 | v | m1 | m7 | ... | m5 | m11
    blocks = [wqkvT[:, 0:128], wqkvT[:, 768:896], wqkvT[:, 1536:2304]]
    for mq, mk in zip((1, 2, 3, 4, 5), (7, 8, 9, 10, 11)):
        blocks.append(wqkvT[:, mq * 128:(mq + 1) * 128])
        blocks.append(wqkvT[:, mk * 128:(mk + 1) * 128])
    wq_host = np.ascontiguousarray(np.concatenate(blocks, axis=1).astype(bf))
    wprojT = np.ascontiguousarray(np.asarray(W_proj, dtype=np.float32).T.astype(bf))
    bp = np.ascontiguousarray(np.asarray(b_proj, dtype=np.float32))
    return [
        {"xT": xT_all[i], "wqkvT": wq_host, "wprojT": wprojT, "bproj": bp}
        for i in range(B)
    ]


def run(x, W_qkv, W_proj, b_proj, trace=False):
    nc = _get_graph()
    in_maps = make_in_maps(x, W_qkv, W_proj, b_proj)
    res = run_bass_kernel_spmd(nc, in_maps, core_ids=list(range(B)), trace=trace)
    out = np.stack([res.results[i]["out"] for i in range(B)], axis=0)
    return out.astype(np.float32, copy=False), res


def kernel(x, W_qkv, W_proj, b_proj, H=None, W=None):
    out, _ = run(x, W_qkv, W_proj, b_proj)
    return out
